# revision 1
# baseline (speedup 1.0000x reference)
"""Trainium2 Bass kernel for nn_FCLSTM: embedding -> custom LSTM-ish recurrence -> select -> linear -> log_softmax.

Self-contained: hardcodes shapes. kernel(**inputs) takes full numpy inputs, returns [64, 2] fp32.
"""
import os
import numpy as np

import concourse.bacc as bacc
import concourse.bass as bass
import concourse.mybir as mybir
from concourse import library_config  # noqa: F401
from concourse.tile import TileContext
from concourse.masks import make_identity
from concourse.bass_utils import run_bass_kernel_spmd

VOCAB, EMBED, HIDDEN, NCLS = 32000, 512, 1024, 2
B, S = 64, 512
NCORES = 8
HC = HIDDEN // NCORES          # 128 per-core H slice for the U table
NVT = VOCAB // 128             # 250 vocab tiles
NEC = EMBED // 128             # 4 embed (contraction) chunks
NKC = HIDDEN // 128            # 8 hidden contraction chunks
TCH = S // 8                   # 64 steps per AllGather time-chunk
TOK = B * S                    # 32768 tokens
F16 = mybir.dt.float16
F32 = mybir.dt.float32
I32 = mybir.dt.int32

_CACHE = {}


def _build(steps=S):
    nc = bacc.Bacc("TRN2", target_bir_lowering=False, debug=False, num_devices=NCORES)

    # ---------- inputs ----------
    embt = nc.dram_tensor("embt", [NVT * NEC * 128, 128], F16, kind="ExternalInput")
    wi = nc.dram_tensor("wi", [EMBED, HC], F16, kind="ExternalInput")
    bi = nc.dram_tensor("bi", [1, HC], F16, kind="ExternalInput")
    wf = nc.dram_tensor("wf", [HIDDEN, HIDDEN], F16, kind="ExternalInput")
    wh = nc.dram_tensor("wh", [HIDDEN, HIDDEN], F16, kind="ExternalInput")
    bf_r = nc.dram_tensor("bf_r", [1, HIDDEN], F16, kind="ExternalInput")
    bh_r = nc.dram_tensor("bh_r", [1, HIDDEN], F16, kind="ExternalInput")
    wo = nc.dram_tensor("wo", [HIDDEN, HIDDEN], F16, kind="ExternalInput")
    bo_r = nc.dram_tensor("bo_r", [1, HIDDEN], F16, kind="ExternalInput")
    wlin = nc.dram_tensor("wlin", [HIDDEN, NCLS], F16, kind="ExternalInput")
    idx = nc.dram_tensor("idx", [128, TOK // 128], I32, kind="ExternalInput")
    selidx = nc.dram_tensor("selidx", [128, 1], I32, kind="ExternalInput")
    out_ext = nc.dram_tensor("out", [B, NCLS], F32, kind="ExternalOutput")

    ntch = (steps + TCH - 1) // TCH  # number of time chunks actually used

    with TileContext(nc) as tc:
        with (
            tc.tile_pool(name="dram", bufs=1, space="DRAM") as dram,
            tc.tile_pool(name="const", bufs=1) as cst,
            tc.tile_pool(name="w", bufs=1) as wpool,
            tc.tile_pool(name="uph", bufs=4) as uph,
            tc.tile_pool(name="upsum", bufs=2, space="PSUM") as upsum,
            tc.tile_pool(name="rec", bufs=3) as rec,
            tc.tile_pool(name="gpsum", bufs=2, space="PSUM") as gpsum,
            tc.tile_pool(name="tpsum", bufs=2, space="PSUM") as tpsum,
        ):
            # ---------- DRAM scratch ----------
            u_dram = dram.tile([VOCAB, HC], F16)
            agin = [dram.tile([B * TCH, HC], F16, name=f"agin{j}") for j in range(ntch)]
            gath = [dram.tile([NCORES * B * TCH, HC], F16, name=f"gath{j}", addr_space="Shared") for j in range(ntch)]
            ring = dram.tile([TOK, HIDDEN], F16)

            # ---------- constants / weights to SBUF ----------
            ones64 = cst.tile([1, 64], F16, tag="ones64")
            nc.vector.memset(ones64[:], 1.0)
            ones128 = cst.tile([1, 128], F16, tag="ones128")
            nc.vector.memset(ones128[:], 1.0)
            ident = cst.tile([64, 64], F16, tag="ident")
            make_identity(nc, ident[:])

            wi_sb = cst.tile([128, NEC * HC], F16, tag="wi")
            for e in range(NEC):
                nc.sync.dma_start(out=wi_sb[:, e * HC:(e + 1) * HC],
                                  in_=wi[e * 128:(e + 1) * 128, :])
            bi_sb = cst.tile([1, HC], F16, tag="bi")
            nc.sync.dma_start(out=bi_sb[:], in_=bi[:])
            bf_sb = cst.tile([1, HIDDEN], F16, tag="bf")
            nc.sync.dma_start(out=bf_sb[:], in_=bf_r[:])
            bh_sb = cst.tile([1, HIDDEN], F16, tag="bh")
            nc.sync.dma_start(out=bh_sb[:], in_=bh_r[:])
            bo_sb = cst.tile([1, HIDDEN], F16, tag="bo")
            nc.sync.dma_start(out=bo_sb[:], in_=bo_r[:])

            wf_sb = wpool.tile([128, NKC * HIDDEN], F16, tag="wf")
            wh_sb = wpool.tile([128, NKC * HIDDEN], F16, tag="wh")
            for k in range(NKC):
                nc.sync.dma_start(out=wf_sb[:, k * HIDDEN:(k + 1) * HIDDEN],
                                  in_=wf[k * 128:(k + 1) * 128, :])
                nc.sync.dma_start(out=wh_sb[:, k * HIDDEN:(k + 1) * HIDDEN],
                                  in_=wh[k * 128:(k + 1) * 128, :])

            # ---------- phase 1: U table  U_c = relu(emb @ WiT_c + bi_c) ----------
            for i in range(NVT):
                et = uph.tile([128, NEC * 128], F16, tag="et")
                src = bass.AP(tensor=embt, offset=i * NEC * 128 * 128,
                              ap=[[128, 128], [128 * 128, NEC], [1, 128]])
                nc.sync.dma_start(out=et[:], in_=src)
                pu = upsum.tile([128, HC], F32, tag="pu")
                for e in range(NEC):
                    nc.tensor.matmul(out=pu[:], lhsT=et[:, e * 128:(e + 1) * 128],
                                     rhs=wi_sb[:, e * HC:(e + 1) * HC],
                                     start=(e == 0), stop=False)
                nc.tensor.matmul(out=pu[:], lhsT=ones128[:], rhs=bi_sb[:],
                                 start=False, stop=True)
                u_sb = uph.tile([128, HC], F16, tag="usb")
                nc.scalar.activation(u_sb[:], pu[:], mybir.ActivationFunctionType.Relu)
                nc.sync.dma_start(out=u_dram[i * 128:(i + 1) * 128, :], in_=u_sb[:])

            # ---------- phase 2: gather inp_c rows (t-major) + phase 3: AllGather ----------
            ng_per_ch = (B * TCH) // 128  # 32 gather calls per time chunk
            ncalls = ntch * ng_per_ch
            idx_all = cst.tile([128, 256], I32, tag="idx_all")
            nc.sync.dma_start(out=idx_all[:, :ncalls], in_=idx[:, 0:ncalls])
            for j in range(ntch):
                for g in range(ng_per_ch):
                    k = j * ng_per_ch + g
                    gt = uph.tile([128, HC], F16, tag="gt")
                    nc.gpsimd.indirect_dma_start(
                        out=gt[:], out_offset=None,
                        in_=u_dram[:, :],
                        in_offset=bass.IndirectOffsetOnAxis(ap=idx_all[:, k:k + 1], axis=0))
                    nc.sync.dma_start(out=agin[j][g * 128:(g + 1) * 128, :], in_=gt[:])
                nc.gpsimd.collective_compute(
                    "AllGather", mybir.AluOpType.bypass,
                    replica_groups=[list(range(NCORES))],
                    ins=[agin[j].opt()], outs=[gath[j].opt()])

            # ---------- phase 4: recurrence ----------
            hT = rec.tile([128, NKC * 64], F16, tag="hT")
            nc.vector.memset(hT[:], 0.0)
            for t in range(steps):
                j, tl = t // TCH, t % TCH
                inp = rec.tile([B, HIDDEN], F16, tag="inp")
                src = bass.AP(tensor=gath[j].tensor, offset=tl * B * HC,
                              ap=[[HC, B], [B * TCH * HC, NCORES], [1, HC]])
                nc.sync.dma_start(out=inp[:], in_=src)
                pg = gpsum.tile([128, HIDDEN], F32, tag="pg")
                # bias rows first (no dependency on h -> PE can run them early)
                for n in range(2):
                    ns = slice(n * 512, (n + 1) * 512)
                    nc.tensor.matmul(out=pg[0:64, ns], lhsT=ones64[:],
                                     rhs=bf_sb[:, ns], start=True, stop=False,
                                     tile_position=(0, 0))
                    nc.tensor.matmul(out=pg[64:128, ns], lhsT=ones64[:],
                                     rhs=bh_sb[:, ns], start=True, stop=False,
                                     tile_position=(0, 64))
                for k in range(NKC - 1):
                    lhs = hT[:, k * 64:(k + 1) * 64]
                    for n in range(2):
                        ns = slice(n * 512, (n + 1) * 512)
                        nc.tensor.matmul(out=pg[0:64, ns], lhsT=lhs,
                                         rhs=wf_sb[:, k * HIDDEN + n * 512:k * HIDDEN + (n + 1) * 512],
                                         start=False, stop=False,
                                         tile_position=(0, 0))
                        nc.tensor.matmul(out=pg[64:128, ns], lhsT=lhs,
                                         rhs=wh_sb[:, k * HIDDEN + n * 512:k * HIDDEN + (n + 1) * 512],
                                         start=False, stop=False,
                                         tile_position=(0, 64))
                # last contraction chunk per half, then act/fma/transpose per half
                k = NKC - 1
                lhs = hT[:, k * 64:(k + 1) * 64]
                sig = rec.tile([B, HIDDEN], F16, tag="sig")
                th = rec.tile([B, HIDDEN], F16, tag="th")
                hnew = rec.tile([B, HIDDEN], F16, tag="hnew")
                pt = tpsum.tile([128, NKC * 64], F16, tag="pt")
                hTn = rec.tile([128, NKC * 64], F16, tag="hT")
                for n in range(2):
                    ns = slice(n * 512, (n + 1) * 512)
                    nc.tensor.matmul(out=pg[0:64, ns], lhsT=lhs,
                                     rhs=wf_sb[:, k * HIDDEN + n * 512:k * HIDDEN + (n + 1) * 512],
                                     start=False, stop=True,
                                     tile_position=(0, 0))
                    nc.tensor.matmul(out=pg[64:128, ns], lhsT=lhs,
                                     rhs=wh_sb[:, k * HIDDEN + n * 512:k * HIDDEN + (n + 1) * 512],
                                     start=False, stop=True,
                                     tile_position=(0, 64))
                    nc.scalar.activation(sig[:, ns], pg[0:64, ns],
                                         mybir.ActivationFunctionType.Sigmoid)
                    nc.scalar.activation(th[:, ns], pg[64:128, ns],
                                         mybir.ActivationFunctionType.Tanh)
                    nc.vector.tensor_mul(out=hnew[:, ns], in0=th[:, ns], in1=inp[:, ns])
                    nc.vector.tensor_add(out=hnew[:, ns], in0=hnew[:, ns], in1=sig[:, ns])
                    for q in range(4):
                        kk = n * 4 + q
                        nc.tensor.transpose(out=pt[:, kk * 64:(kk + 1) * 64],
                                            in_=hnew[:, kk * 128:(kk + 1) * 128],
                                            identity=ident[:])
                    nc.vector.tensor_copy(out=hTn[:, n * 256:(n + 1) * 256],
                                          in_=pt[:, n * 256:(n + 1) * 256])
                nc.sync.dma_start(out=ring[t * B:(t + 1) * B, :], in_=hnew[:])
                hT = hTn

            # ---------- phase 5: select + linear + log_softmax ----------
            six = cst.tile([128, 1], I32, tag="six")
            nc.sync.dma_start(out=six[:], in_=selidx[:])
            hsel = cst.tile([128, HIDDEN], F16, tag="hsel")
            nc.gpsimd.indirect_dma_start(
                out=hsel[:], out_offset=None,
                in_=ring[:, :],
                in_offset=bass.IndirectOffsetOnAxis(ap=six[:, :1], axis=0))
            # transpose hsel[0:64] -> hselT chunks
            pt2 = tpsum.tile([128, NKC * 64], F16, tag="pt")
            for k in range(NKC):
                nc.tensor.transpose(out=pt2[:, k * 64:(k + 1) * 64],
                                    in_=hsel[0:64, k * 128:(k + 1) * 128],
                                    identity=ident[:])
            hselT = cst.tile([128, NKC * 64], F16, tag="hselT")
            nc.vector.tensor_copy(out=hselT[:], in_=pt2[:])
            # lin = hsel @ WoT + bo
            wo_sb = wpool.tile([128, NKC * HIDDEN], F16, tag="wo")
            for k in range(NKC):
                nc.sync.dma_start(out=wo_sb[:, k * HIDDEN:(k + 1) * HIDDEN],
                                  in_=wo[k * 128:(k + 1) * 128, :])
            pl = gpsum.tile([64, HIDDEN], F32, tag="pg")
            for k in range(NKC):
                for n in range(2):
                    ns = slice(n * 512, (n + 1) * 512)
                    nc.tensor.matmul(out=pl[:, ns], lhsT=hselT[:, k * 64:(k + 1) * 64],
                                     rhs=wo_sb[:, k * HIDDEN + n * 512:k * HIDDEN + (n + 1) * 512],
                                     start=(k == 0), stop=False)
            for n in range(2):
                ns = slice(n * 512, (n + 1) * 512)
                nc.tensor.matmul(out=pl[:, ns], lhsT=ones64[:], rhs=bo_sb[:, ns],
                                 start=False, stop=True)
            lin = cst.tile([64, HIDDEN], F16, tag="lin")
            nc.vector.tensor_copy(out=lin[:], in_=pl[:])
            pt3 = tpsum.tile([128, NKC * 64], F16, tag="pt")
            for k in range(NKC):
                nc.tensor.transpose(out=pt3[:, k * 64:(k + 1) * 64],
                                    in_=lin[:, k * 128:(k + 1) * 128],
                                    identity=ident[:])
            linT = cst.tile([128, NKC * 64], F16, tag="linT")
            nc.vector.tensor_copy(out=linT[:], in_=pt3[:])
            wl_sb = cst.tile([128, NKC * NCLS], F16, tag="wl")
            for k in range(NKC):
                nc.sync.dma_start(out=wl_sb[:, k * NCLS:(k + 1) * NCLS],
                                  in_=wlin[k * 128:(k + 1) * 128, :])
            pz = upsum.tile([64, NCLS], F32, tag="pu")
            for k in range(NKC):
                nc.tensor.matmul(out=pz[:], lhsT=linT[:, k * 64:(k + 1) * 64],
                                 rhs=wl_sb[:, k * NCLS:(k + 1) * NCLS],
                                 start=(k == 0), stop=(k == NKC - 1))
            # log_softmax over the 2 classes (free axis)
            m = cst.tile([64, 1], F32, tag="m")
            nc.vector.tensor_reduce(out=m[:], in_=pz[:], axis=mybir.AxisListType.X,
                                    op=mybir.AluOpType.max)
            xm = cst.tile([64, NCLS], F32, tag="xm")
            nc.vector.tensor_scalar(out=xm[:], in0=pz[:], scalar1=m[:], scalar2=None,
                                    op0=mybir.AluOpType.subtract)
            esum = cst.tile([64, 1], F32, tag="esum")
            ex = cst.tile([64, NCLS], F32, tag="ex")
            nc.scalar.activation(ex[:], xm[:], mybir.ActivationFunctionType.Exp,
                                 accum_out=esum[:])
            lns = cst.tile([64, 1], F32, tag="lns")
            nc.scalar.activation(lns[:], esum[:], mybir.ActivationFunctionType.Ln)
            res = cst.tile([64, NCLS], F32, tag="res")
            nc.vector.tensor_scalar(out=res[:], in0=xm[:], scalar1=lns[:], scalar2=None,
                                    op0=mybir.AluOpType.subtract)
            nc.sync.dma_start(out=out_ext[:, :], in_=res[:])

    nc.compile()
    return nc


def _prep(x, lengths, emb, W_i, b_i, W_f, b_f, W_h, b_h, W_o, b_o, W_lin, b_lin,
          steps=S):
    f16 = np.float16
    embT = emb.T.astype(f16)  # [512, 32000]
    # tile-major layout: tile (i, e) = embT[e*128:(e+1)*128, i*128:(i+1)*128]
    et = embT.reshape(NEC, 128, NVT, 128).transpose(2, 0, 1, 3).reshape(NVT * NEC * 128, 128)
    x_tm = np.ascontiguousarray(x.T)  # [S, B] t-major
    idx_tm = np.ascontiguousarray(x_tm.reshape(TOK // 128, 128).T).astype(np.int32)  # [128, 256] col-major
    sel = ((lengths.astype(np.int64) - 1) * B + np.arange(B)).astype(np.int32)
    selpad = np.zeros((128, 1), np.int32)
    selpad[:B, 0] = sel
    maps = []
    for c in range(NCORES):
        hsl = slice(c * HC, (c + 1) * HC)
        maps.append({
            "embt": np.ascontiguousarray(et),
            "wi": np.ascontiguousarray(W_i[hsl, :].T.astype(f16)),
            "bi": b_i[None, hsl].astype(f16),
            "wf": np.ascontiguousarray(W_f.T.astype(f16)),
            "wh": np.ascontiguousarray(W_h.T.astype(f16)),
            "bf_r": b_f[None, :].astype(f16),
            "bh_r": b_h[None, :].astype(f16),
            "wo": np.ascontiguousarray(W_o.T.astype(f16)),
            "bo_r": b_o[None, :].astype(f16),
            "wlin": np.ascontiguousarray(W_lin.T.astype(f16)),
            "idx": idx_tm,
            "selidx": selpad,
        })
    return maps


def _run(inputs, steps=S, trace=False):
    key = steps
    if key not in _CACHE:
        _CACHE[key] = _build(steps)
    nc = _CACHE[key]
    maps = _prep(**inputs, steps=steps)
    res = run_bass_kernel_spmd(nc, maps, core_ids=list(range(NCORES)), trace=trace)
    return res


def kernel(**inputs) -> np.ndarray:
    res = _run(inputs, steps=S, trace=False)
    return res.results[0]["out"]


if __name__ == "__main__":
    steps = int(os.environ.get("KSTEPS", "8"))
    rng = np.random.default_rng(0)
    x = rng.integers(0, VOCAB, size=(B, S)).astype(np.int64)
    lengths = rng.integers(1, steps + 1, size=(B,)).astype(np.int64)
    lengths[0] = steps
    s_e, s_h = 1 / np.sqrt(EMBED), 1 / np.sqrt(HIDDEN)
    ins = dict(
        x=x, lengths=lengths,
        emb=rng.normal(size=(VOCAB, EMBED)).astype(np.float32),
        W_i=rng.uniform(-s_e, s_e, (HIDDEN, EMBED)).astype(np.float32),
        b_i=rng.uniform(-s_e, s_e, (HIDDEN,)).astype(np.float32),
        W_f=rng.uniform(-s_h, s_h, (HIDDEN, HIDDEN)).astype(np.float32),
        b_f=rng.uniform(-s_h, s_h, (HIDDEN,)).astype(np.float32),
        W_h=rng.uniform(-s_h, s_h, (HIDDEN, HIDDEN)).astype(np.float32),
        b_h=rng.uniform(-s_h, s_h, (HIDDEN,)).astype(np.float32),
        W_o=rng.uniform(-s_h, s_h, (HIDDEN, HIDDEN)).astype(np.float32),
        b_o=rng.uniform(-s_h, s_h, (HIDDEN,)).astype(np.float32),
        W_lin=rng.uniform(-s_h, s_h, (NCLS, HIDDEN)).astype(np.float32),
        b_lin=np.zeros((NCLS,), np.float32),
    )
    # numpy reference (on truncated steps)
    def npref(steps):
        e = ins["emb"][x]  # [B, S, E]
        h = np.zeros((B, HIDDEN), np.float32)
        outs = np.zeros((steps, B, HIDDEN), np.float32)
        for t in range(steps):
            et_ = e[:, t, :]
            inp = np.maximum(et_ @ ins["W_i"].T + ins["b_i"], 0)
            hf = 1 / (1 + np.exp(-(h @ ins["W_f"].T + ins["b_f"])))
            hh = np.tanh(h @ ins["W_h"].T + ins["b_h"])
            h = hf + hh * inp
            outs[t] = h
        li = outs[lengths - 1, np.arange(B)]
        lin = li @ ins["W_o"].T + ins["b_o"]
        lg = lin @ ins["W_lin"].T + ins["b_lin"]
        lg = lg - lg.max(1, keepdims=True)
        return lg - np.log(np.exp(lg).sum(1, keepdims=True))

    expected = npref(steps)
    res = _run(ins, steps=steps, trace=False)
    got = res.results[0]["out"]
    err = np.linalg.norm(got - expected) / np.linalg.norm(expected)
    print("expected[:3]:", expected[:3])
    print("got[:3]:", got[:3])
    print("rel_err:", err)

